# revision 8
# baseline (speedup 1.0000x reference)
"""GAT (graph attention) layer on 8 TRN2 NeuronCores — v2.

Algorithm (mathematically equal to the reference):
  proj = in_feat @ W_proj;  src_s = proj @ a_src;  tau = proj @ a_tgt
  per edge e=(s,t):  score_e = exp(leakyrelu(src_s[s] + tau[t]) - SHIFT)
  out[t] = (sum_e score_e * proj[s]) / (sum_e score_e) + bias
The reference's global-max shift is replaced by the constant SHIFT=16
(numerator/denominator scale identically).  exp(leakyrelu(x) - S) is
computed as max(exp(x-S), exp(0.2x-S)) — two ACT exps + one DVE max.

Sharding: edges sharded by TARGET node; each core owns a disjoint output
slice, no collectives.  Per core, targets are packed into 128-target
blocks; each block's segment sums (softmax denominator + weighted
feature sum) accumulate in PSUM via one-hot matmuls.

Cost-model-driven design notes:
 - dma_gather's engine cost is output-free-size x 0.833ns, so all gather
   APs are declared int64 (byte-mover; halves the Pool cost vs f32).
 - Bulk DRAM->SBUF loads (x slabs, index arrays) are sequential-index
   gathers: far cheaper on the shared DMA-engine resource than dma_start.
 - The one-hot matrix S is gathered from a constant identity table
   (Pool) instead of DVE is_equal — frees the DVE bottleneck.
 - The proj table is stored p-major (row = (n%128)*NT + n//128) so
   phase-1 writes are big contiguous descriptors (no 2x sub-512B
   penalty); gathers split at partition 64 for int16 indices.
 - Scores (src_s|tau) live in a separate 256B-stride table (32B rows
   written); blocks hold targets of a single p-half so the per-edge tau
   gather hits one table.
"""
import sys
sys.path.insert(0, "/opt/trn_rl_repo")
import numpy as np

import concourse.bass as bass
import concourse.bacc as bacc
import concourse.mybir as mybir
import concourse.tile as tile
from concourse._compat import cdiv

P = 128
N_NODES = 50000
N_CORES = 8
D = 128
H = 4
NT = cdiv(N_NODES, P)               # 391 node tiles
NPAD = NT * P                       # 50048
SHIFT = 16.0
EPS = 1e-16
PSPLIT = 64                         # partition split for int16 p-major idx
ROWS_LO = PSPLIT * NT               # 25024
ROWS_HI = (P - PSPLIT) * NT
TPC = N_NODES // N_CORES            # 6250 targets per core
XSLAB = 49                          # node tiles per x-slab input
NXS = cdiv(NT, XSLAB)               # 4
PK = 4                              # node tiles per phase-1 psum group
WSLAB = 48                          # node tiles per phase-1 table write

_cache = {}

CFG = {
    "p1_copy_act": 3,    # of every 3 groups, how many proj copies on ACT
    "acc_bufs": 4,
    "g_bufs": 4,
    "wk_bufs": 4,
}


def _build(nb_lo, nb_hi, k_plo, k_phi, with_bias):
    nc = bacc.Bacc("TRN2", target_bir_lowering=False, debug=False)
    f32, bf16 = mybir.dt.float32, mybir.dt.bfloat16
    i16, i64 = mybir.dt.int16, mybir.dt.int64

    NBLK = nb_lo + nb_hi
    T_B = k_plo + k_phi                 # edge tiles per block
    NIDX = T_B * P
    IW = T_B * 8                        # wrapped idx cols per block
    IWPAD = cdiv(NBLK * IW * 2, 256) * 128  # idx table cols, 256B-mult rows

    # ---- inputs ----
    xs_d = [nc.dram_tensor(f"xs{i}", [P, XSLAB * P], bf16, kind="ExternalInput")
            for i in range(NXS)]
    W_d = nc.dram_tensor("W", [P, 136], bf16, kind="ExternalInput")
    ident_d = nc.dram_tensor("ident", [144, 32], i64, kind="ExternalInput")
    pidx_d = nc.dram_tensor("pidx", [P, IWPAD], i16, kind="ExternalInput")
    tidx_d = nc.dram_tensor("tidx", [P, IWPAD], i16, kind="ExternalInput")
    sidx_d = nc.dram_tensor("sidx", [P, IWPAD], i16, kind="ExternalInput")
    seq_d = nc.dram_tensor("seq", [P, 16], i16, kind="ExternalInput")
    if with_bias:
        bias_d = nc.dram_tensor("bias", [1, D], f32, kind="ExternalInput")
    out_d = nc.dram_tensor("out", [NBLK * P, D], f32, kind="ExternalOutput")

    # ---- tables (device-built) ----
    pt_lo = nc.dram_tensor("pt_lo", [ROWS_LO, 32], i64)
    pt_hi = nc.dram_tensor("pt_hi", [ROWS_HI, 32], i64)
    st_lo = nc.dram_tensor("st_lo", [ROWS_LO, 32], i64)
    st_hi = nc.dram_tensor("st_hi", [ROWS_HI, 32], i64)

    with tile.TileContext(nc) as tc:
        with (
            tc.tile_pool(name="const", bufs=1) as cp,
            tc.tile_pool(name="p1x", bufs=3) as p1x,
            tc.tile_pool(name="p1w", bufs=2) as p1w,
            tc.tile_pool(name="p1ps", bufs=2, space="PSUM") as p1ps,
            tc.tile_pool(name="g", bufs=CFG["g_bufs"]) as g,
            tc.tile_pool(name="wk", bufs=CFG["wk_bufs"]) as wk,
            tc.tile_pool(name="acc", bufs=CFG["acc_bufs"], space="PSUM") as accp,
            tc.tile_pool(name="ep", bufs=3) as ep,
        ):
            from concourse.library_config import mlp
            nc.gpsimd.load_library(mlp)

            seq = cp.tile([P, 16], i16)
            nc.sync.dma_start(seq[:], seq_d[:])
            W_sb = cp.tile([P, 136], bf16)
            nc.sync.dma_start(W_sb[:], W_d[:])
            nshift = cp.tile([P, 1], f32)
            nc.gpsimd.memset(nshift[:], -SHIFT)
            sc02 = cp.tile([P, 1], f32)
            nc.gpsimd.memset(sc02[:], 0.2)

            def bulk_load(dst_ap, src_t, n_i64):
                # dst[p, :] = src_t[p, :] via 128-row sequential gather
                nc.gpsimd.dma_gather(
                    dst_ap.rearrange("p (k c) -> p k c", k=1),
                    src_t, seq[:, :8], P, P, n_i64, single_packet=False)

            pidx = cp.tile([P, IWPAD], i16)
            tidx = cp.tile([P, IWPAD], i16)
            sidx = cp.tile([P, IWPAD], i16)
            bulk_load(pidx[:].bitcast(i64), pidx_d[:].bitcast(i64), IWPAD // 4)
            bulk_load(tidx[:].bitcast(i64), tidx_d[:].bitcast(i64), IWPAD // 4)
            bulk_load(sidx[:].bitcast(i64), sidx_d[:].bitcast(i64), IWPAD // 4)
            if with_bias:
                ones_row = cp.tile([1, P], f32)
                nc.gpsimd.memset(ones_row[:], 1.0)
                bias_row = cp.tile([1, D], f32)
                nc.sync.dma_start(bias_row[:], bias_d[:])
                bias_ps = accp.tile([P, D], f32, tag="init")
                nc.tensor.matmul(out=bias_ps[:], lhsT=ones_row[:], rhs=bias_row[:],
                                 start=True, stop=True)
                bias_mat = cp.tile([P, D], f32)
                nc.vector.tensor_copy(out=bias_mat[:], in_=bias_ps[:])

            # ---- phase 1: projection + scores -> tables ----
            xs = []
            for i in range(NXS):
                xt = p1x.tile([P, XSLAB * P], bf16, tag="xs")
                bulk_load(xt[:].bitcast(i64), xs_d[i][:].bitcast(i64),
                          XSLAB * P // 4)
                xs.append(xt)

            n_wslab = cdiv(NT, WSLAB)
            gi = 0
            for ws in range(n_wslab):
                base = ws * WSLAB
                w = min(WSLAB, NT - base)
                prow = p1w.tile([P, WSLAB * D], bf16, tag="prow")
                srow = p1w.tile([P, WSLAB * 8], f32, tag="srow")
                prow_r = prow[:].rearrange("p (j c) -> p j c", j=WSLAB)
                srow_r = srow[:].rearrange("p (j c) -> p j c", j=WSLAB)
                for g0 in range(0, w, PK):
                    k = min(PK, w - g0)
                    ps = p1ps.tile([P, PK * 256], f32, tag="p1")
                    for j in range(k):
                        nt = base + g0 + j
                        xt = xs[nt // XSLAB]
                        o = (nt % XSLAB) * P
                        nc.tensor.matmul(out=ps[:, j * 256:j * 256 + 136],
                                         lhsT=xt[:, o:o + P],
                                         rhs=W_sb[:], start=True, stop=True)
                    ps_r = ps[:].rearrange("p (j c) -> p j c", j=PK)[:, :k, :]
                    ceng = nc.scalar if (gi % 3) < CFG["p1_copy_act"] else nc.vector
                    gi += 1
                    if ceng is nc.scalar:
                        nc.scalar.activation(
                            out=prow_r[:, g0:g0 + k, :], in_=ps_r[:, :, 0:D],
                            func=mybir.ActivationFunctionType.Copy)
                    else:
                        nc.vector.tensor_copy(
                            out=prow_r[:, g0:g0 + k, :], in_=ps_r[:, :, 0:D])
                    nc.vector.tensor_copy(
                        out=srow_r[:, g0:g0 + k, :], in_=ps_r[:, :, D:D + 8])
                pr = prow_r[:, :w, :]
                sr = srow_r[:, :w, :]
                nc.sync.dma_start(
                    pt_lo[:].bitcast(bf16).rearrange(
                        "(p nt) c -> p nt c", p=PSPLIT)[:, base:base + w, :],
                    pr[0:PSPLIT])
                nc.sync.dma_start(
                    pt_hi[:].bitcast(bf16).rearrange(
                        "(p nt) c -> p nt c", p=P - PSPLIT)[:, base:base + w, :],
                    pr[PSPLIT:P])
                nc.sync.dma_start(
                    st_lo[:].bitcast(f32).rearrange(
                        "(p nt) c -> p nt c", p=PSPLIT)[:, base:base + w, 0:8],
                    sr[0:PSPLIT])
                nc.sync.dma_start(
                    st_hi[:].bitcast(f32).rearrange(
                        "(p nt) c -> p nt c", p=P - PSPLIT)[:, base:base + w, 0:8],
                    sr[PSPLIT:P])

            # ---- phase 2: per 128-target block ----
            for b in range(NBLK):
                st_t = st_lo if b < nb_lo else st_hi
                gi0 = b * IW
                rows = g.tile([P, T_B * 32], i64, tag="grow")
                srcs = g.tile([P, T_B * 32], i64, tag="gsrc")
                taut = g.tile([P, T_B * 32], i64, tag="gtau")
                Sg = g.tile([P, T_B * 32], i64, tag="gS")
                if k_plo:
                    nc.gpsimd.dma_gather(
                        rows[:, :k_plo * 32].rearrange("p (k c) -> p k c", k=k_plo),
                        pt_lo[:], pidx[:, gi0:gi0 + k_plo * 8],
                        k_plo * P, k_plo * P, 32, single_packet=False)
                    nc.gpsimd.dma_gather(
                        srcs[:, :k_plo * 32].rearrange("p (k c) -> p k c", k=k_plo),
                        st_lo[:], pidx[:, gi0:gi0 + k_plo * 8],
                        k_plo * P, k_plo * P, 32, single_packet=False)
                if k_phi:
                    nc.gpsimd.dma_gather(
                        rows[:, k_plo * 32:].rearrange("p (k c) -> p k c", k=k_phi),
                        pt_hi[:], pidx[:, gi0 + k_plo * 8:gi0 + IW],
                        k_phi * P, k_phi * P, 32, single_packet=False)
                    nc.gpsimd.dma_gather(
                        srcs[:, k_plo * 32:].rearrange("p (k c) -> p k c", k=k_phi),
                        st_hi[:], pidx[:, gi0 + k_plo * 8:gi0 + IW],
                        k_phi * P, k_phi * P, 32, single_packet=False)
                nc.gpsimd.dma_gather(
                    taut[:].rearrange("p (k c) -> p k c", k=T_B),
                    st_t[:], tidx[:, gi0:gi0 + IW],
                    NIDX, NIDX, 32, single_packet=False)
                nc.gpsimd.dma_gather(
                    Sg[:].rearrange("p (k c) -> p k c", k=T_B),
                    ident_d[:], sidx[:, gi0:gi0 + IW],
                    NIDX, NIDX, 32, single_packet=False)

                srcs_f = srcs[:].bitcast(f32).rearrange("p (j c) -> p j c", j=T_B)
                taut_f = taut[:].bitcast(f32).rearrange("p (j c) -> p j c", j=T_B)
                xb = wk.tile([P, T_B * H], f32, tag="xb")
                xb_r = xb[:].rearrange("p (j h) -> p j h", j=T_B)
                nc.vector.tensor_tensor(
                    out=xb_r, in0=srcs_f[:, :, 0:H], in1=taut_f[:, :, H:2 * H],
                    op=mybir.AluOpType.add)
                wide = wk.tile([P, T_B * (D + H)], bf16, tag="wide")
                wide_r = wide[:].rearrange("p (j c) -> p j c", j=T_B)
                e2 = wk.tile([P, T_B * H], bf16, tag="e2")
                e2_r = e2[:].rearrange("p (j h) -> p j h", j=T_B)
                nc.scalar.activation(
                    out=wide_r[:, :, D:], in_=xb_r,
                    func=mybir.ActivationFunctionType.Exp, bias=nshift[:])
                nc.scalar.activation(
                    out=e2_r, in_=xb_r,
                    func=mybir.ActivationFunctionType.Exp, bias=nshift[:],
                    scale=sc02[:])
                nc.vector.tensor_tensor(
                    out=wide_r[:, :, D:], in0=wide_r[:, :, D:], in1=e2_r,
                    op=mybir.AluOpType.max)
                nc.vector.tensor_tensor(
                    out=wide_r[:, :, :D].rearrange("p j (r h) -> p j r h", h=H),
                    in0=rows[:].bitcast(bf16).rearrange("p (j c) -> p j c", j=T_B)
                        .rearrange("p j (r h) -> p j r h", h=H),
                    in1=wide_r[:, :, D:].unsqueeze(2).to_broadcast([P, T_B, 32, H]),
                    op=mybir.AluOpType.mult)
                acc = accp.tile([P, D + H], f32, tag="acc")
                Sg_b = Sg[:].bitcast(bf16).rearrange("p (j c) -> p j c", j=T_B)
                for j in range(T_B):
                    nc.tensor.matmul(
                        out=acc[:], lhsT=Sg_b[:, j, :],
                        rhs=wide[:, j * (D + H):(j + 1) * (D + H)],
                        start=(j == 0), stop=(j == T_B - 1))
                den = ep.tile([P, H], f32, tag="den")
                nc.scalar.activation(out=den[:], in_=acc[:, D:],
                                     func=mybir.ActivationFunctionType.Copy,
                                     bias=float(EPS))
                recip = ep.tile([P, H], f32, tag="recip")
                nc.vector.reciprocal(recip[:], den[:])
                out_sb = ep.tile([P, D], f32, tag="outsb")
                nc.vector.tensor_tensor(
                    out=out_sb[:].rearrange("p (h r) -> p r h", h=H),
                    in0=acc[:, :D].rearrange("p (r h) -> p r h", h=H),
                    in1=recip[:].unsqueeze(1).to_broadcast([P, 32, H]),
                    op=mybir.AluOpType.mult)
                if with_bias:
                    nc.vector.tensor_tensor(
                        out=out_sb[:], in0=out_sb[:], in1=bias_mat[:],
                        op=mybir.AluOpType.add)
                nc.sync.dma_start(out_d[b * P:(b + 1) * P, :], out_sb[:])

    nc.compile()
    return nc


def _wrap16(seg):
    """dma_gather idx layout: entry i at [i%16, i//16], replicated x8."""
    n = len(seg)
    w = seg.reshape(n // 16, 16).T
    return np.tile(w, (8, 1))


def _prep_host(in_feat, edge_ind, W_proj, a_src, a_tgt, bias):
    import ml_dtypes
    bfd = ml_dtypes.bfloat16
    src = np.asarray(edge_ind[0]).astype(np.int64)
    tgt = np.asarray(edge_ind[1]).astype(np.int64)
    x = np.asarray(in_feat, np.float32)
    W = np.asarray(W_proj, np.float32)
    a_src = np.asarray(a_src, np.float32).reshape(H, 32)
    a_tgt = np.asarray(a_tgt, np.float32).reshape(H, 32)
    bias = np.asarray(bias, np.float32).reshape(-1)

    # W_ext: [W head-interleaved (col r*4+h) | W@a_src_h | W@a_tgt_h], bf16
    Wb = W.astype(bfd).astype(np.float32)
    perm = np.arange(D).reshape(H, 32).T.reshape(-1)   # new col r*4+h = old h*32+r
    W_ext = np.zeros((P, 136), np.float32)
    W_ext[:, :D] = Wb[:, perm]
    for h in range(H):
        sel = np.zeros((D,), np.float32)
        sel[h * 32:(h + 1) * 32] = a_src[h]
        W_ext[:, D + h] = Wb @ sel
        sel = np.zeros((D,), np.float32)
        sel[h * 32:(h + 1) * 32] = a_tgt[h]
        W_ext[:, D + H + h] = Wb @ sel

    xT = np.zeros((P, NPAD), np.float32)
    xT[:, :N_NODES] = x.T
    xs_in = {}
    for i in range(NXS):
        sl = np.zeros((P, XSLAB * P), bfd)
        w = min(XSLAB * P, NPAD - i * XSLAB * P)
        sl[:, :w] = xT[:, i * XSLAB * P:i * XSLAB * P + w].astype(bfd)
        xs_in[f"xs{i}"] = sl

    ident = np.zeros((144, P), bfd)
    for q in range(P):
        ident[q, q] = 1.0

    # ---- edge partitioning ----
    core = tgt // TPC
    p_of_t = tgt % P
    t_is_lo = p_of_t < PSPLIT
    src_is_lo = (src % P) < PSPLIT

    deg_lo = np.bincount(tgt[src_is_lo], minlength=N_NODES)
    deg_hi = np.bincount(tgt[~src_is_lo], minlength=N_NODES)
    blk_of = np.full(N_NODES, -1, np.int32)
    tin_of = np.zeros(N_NODES, np.int32)
    nb_lo = nb_hi = 0
    for c in range(N_CORES):
        ids_all = np.arange(c * TPC, (c + 1) * TPC)
        nb_lo = max(nb_lo, cdiv(int(((ids_all % P) < PSPLIT).sum()), P))
        nb_hi = max(nb_hi, cdiv(int(((ids_all % P) >= PSPLIT).sum()), P))
    for c in range(N_CORES):
        ids_all = np.arange(c * TPC, (c + 1) * TPC)
        for half, nb, b0 in ((0, nb_lo, 0), (1, nb_hi, nb_lo)):
            sel = (ids_all % P) < PSPLIT if half == 0 else (ids_all % P) >= PSPLIT
            ids = ids_all[sel]
            order = np.argsort(-(deg_lo[ids] + deg_hi[ids]), kind="stable")
            loads_l = np.zeros(nb, np.int64)
            loads_h = np.zeros(nb, np.int64)
            fill = np.zeros(nb, np.int32)
            for t in ids[order]:
                cand = np.nonzero(fill < P)[0]
                j = cand[np.argmin(np.maximum(loads_l[cand] + deg_lo[t],
                                              loads_h[cand] + deg_hi[t])
                                   + 0.001 * fill[cand])]
                blk_of[t] = b0 + j
                tin_of[t] = fill[j]
                fill[j] += 1
                loads_l[j] += deg_lo[t]
                loads_h[j] += deg_hi[t]
    NBLK = nb_lo + nb_hi
    blk = blk_of[tgt]
    tin = tin_of[tgt]

    key = core * NBLK + blk
    n_lo_e = np.bincount(key[src_is_lo], minlength=N_CORES * NBLK)
    n_hi_e = np.bincount(key[~src_is_lo], minlength=N_CORES * NBLK)
    k_plo = max(1, cdiv(int(n_lo_e.max()), P))
    k_phi = max(1, cdiv(int(n_hi_e.max()), P))
    T_B = k_plo + k_phi
    IW = T_B * 8
    IWPAD = cdiv(NBLK * IW * 2, 256) * 128

    prow_id = (src % P - np.where(src_is_lo, 0, PSPLIT)) * NT + src // P
    trow_id = (tgt % P - np.where(t_is_lo, 0, PSPLIT)) * NT + tgt // P

    seq = _wrap16(np.concatenate([np.arange(P, dtype=np.int16),
                                  np.zeros(P, np.int16)]))[:, :16]
    with_bias = bool(np.any(bias != 0.0))
    shared = {**xs_in, "W": W_ext.astype(bfd), "ident": ident.view(np.int64),
              "seq": seq}
    if with_bias:
        shared["bias"] = bias.reshape(1, D)

    core_inputs = []
    out_perm = np.full((N_CORES, NBLK * P), -1, np.int64)
    for c in range(N_CORES):
        ids_all = np.arange(c * TPC, (c + 1) * TPC)
        for t in ids_all:
            out_perm[c, blk_of[t] * P + tin_of[t]] = t
        m = core == c
        cs_p, cb, ct = prow_id[m], blk[m], tin[m]
        ct_row = trow_id[m]
        clo = src_is_lo[m]
        pidx = np.zeros((NBLK, T_B * P), np.int16)
        t16 = np.zeros((NBLK, T_B * P), np.int16)
        s16 = np.full((NBLK, T_B * P), 128, np.int16)   # pad -> zero one-hot row
        for b in range(NBLK):
            mb = cb == b
            lo_sel = mb & clo
            hi_sel = mb & ~clo
            nl, nh = int(lo_sel.sum()), int(hi_sel.sum())
            pidx[b, :nl] = cs_p[lo_sel].astype(np.int16)
            pidx[b, k_plo * P:k_plo * P + nh] = cs_p[hi_sel].astype(np.int16)
            t16[b, :nl] = ct_row[lo_sel].astype(np.int16)
            t16[b, k_plo * P:k_plo * P + nh] = ct_row[hi_sel].astype(np.int16)
            s16[b, :nl] = ct[lo_sel].astype(np.int16)
            s16[b, k_plo * P:k_plo * P + nh] = ct[hi_sel].astype(np.int16)
        pw = np.zeros((P, IWPAD), np.int16)
        tw = np.zeros((P, IWPAD), np.int16)
        sw = np.zeros((P, IWPAD), np.int16)
        for b in range(NBLK):
            pw[:, b * IW:b * IW + k_plo * 8] = _wrap16(pidx[b, :k_plo * P])
            pw[:, b * IW + k_plo * 8:(b + 1) * IW] = _wrap16(pidx[b, k_plo * P:])
            tw[:, b * IW:(b + 1) * IW] = _wrap16(t16[b])
            sw[:, b * IW:(b + 1) * IW] = _wrap16(s16[b])
        core_inputs.append({**shared, "pidx": pw, "tidx": tw, "sidx": sw})
    return (nb_lo, nb_hi, k_plo, k_phi, with_bias), core_inputs, out_perm


def kernel(in_feat, edge_ind, edge_len, W_proj, a_src, a_tgt, bias):
    kkey, core_inputs, out_perm = _prep_host(in_feat, edge_ind, W_proj,
                                             a_src, a_tgt, bias)
    if kkey not in _cache:
        _cache[kkey] = _build(*kkey)
    nc = _cache[kkey]

    from concourse.bass_utils import run_bass_kernel_spmd
    res = run_bass_kernel_spmd(nc, core_inputs, list(range(N_CORES)))

    out = np.zeros((N_NODES, D), np.float32)
    for c in range(N_CORES):
        o = res.results[c]["out"]
        valid = out_perm[c] >= 0
        out[out_perm[c][valid]] = o[valid]
    return out


# revision 9
# speedup vs baseline: 1.0684x; 1.0684x over previous
"""GAT (graph attention) layer on 8 TRN2 NeuronCores — v2.

Algorithm (mathematically equal to the reference):
  proj = in_feat @ W_proj;  src_s = proj @ a_src;  tau = proj @ a_tgt
  per edge e=(s,t):  score_e = exp(leakyrelu(src_s[s] + tau[t]) - SHIFT)
  out[t] = (sum_e score_e * proj[s]) / (sum_e score_e) + bias
The reference's global-max shift is replaced by the constant SHIFT=16
(numerator/denominator scale identically).  exp(leakyrelu(x) - S) is
computed as max(exp(x-S), exp(0.2x-S)) — two ACT exps + one DVE max.

Sharding: edges sharded by TARGET node; each core owns a disjoint output
slice, no collectives.  Per core, targets are packed into 128-target
blocks; each block's segment sums (softmax denominator + weighted
feature sum) accumulate in PSUM via one-hot matmuls.

Cost-model-driven design notes:
 - dma_gather's engine cost is output-free-size x 0.833ns, so all gather
   APs are declared int64 (byte-mover; halves the Pool cost vs f32).
 - Bulk DRAM->SBUF loads (x slabs, index arrays) are sequential-index
   gathers: far cheaper on the shared DMA-engine resource than dma_start.
 - The one-hot matrix S is gathered from a constant identity table
   (Pool) instead of DVE is_equal — frees the DVE bottleneck.
 - The proj table is stored p-major (row = (n%128)*NT + n//128) so
   phase-1 writes are big contiguous descriptors (no 2x sub-512B
   penalty); gathers split at partition 64 for int16 indices.
 - Scores (src_s|tau) live in a separate 256B-stride table (32B rows
   written); blocks hold targets of a single p-half so the per-edge tau
   gather hits one table.
"""
import sys
sys.path.insert(0, "/opt/trn_rl_repo")
import numpy as np

import concourse.bass as bass
import concourse.bacc as bacc
import concourse.mybir as mybir
import concourse.tile as tile
from concourse._compat import cdiv

P = 128
N_NODES = 50000
N_CORES = 8
D = 128
H = 4
NT = cdiv(N_NODES, P)               # 391 node tiles
NPAD = NT * P                       # 50048
SHIFT = 16.0
EPS = 1e-16
PSPLIT = 64                         # partition split for int16 p-major idx
ROWS_LO = PSPLIT * NT               # 25024
ROWS_HI = (P - PSPLIT) * NT
TPC = N_NODES // N_CORES            # 6250 targets per core
XSLAB = 49                          # node tiles per x-slab input
NXS = cdiv(NT, XSLAB)               # 4
PK = 4                              # node tiles per phase-1 psum group
WSLAB = 48                          # node tiles per phase-1 table write

_cache = {}

CFG = {
    "p1_copy_act": 3,    # of every 3 groups, how many proj copies on ACT
    "acc_bufs": 4,
    "g_bufs": 4,
    "wk_bufs": 4,
}


def _build(nb_lo, nb_hi, k_plo, k_phi, with_bias):
    nc = bacc.Bacc("TRN2", target_bir_lowering=False, debug=False)
    f32, bf16 = mybir.dt.float32, mybir.dt.bfloat16
    i16, i64 = mybir.dt.int16, mybir.dt.int64

    NBLK = nb_lo + nb_hi
    T_B = k_plo + k_phi                 # edge tiles per block
    NIDX = T_B * P
    IW = T_B * 8                        # wrapped idx cols per block
    IWPAD = cdiv(NBLK * IW * 2, 256) * 128  # idx table cols, 256B-mult rows

    # ---- inputs ----
    xs_d = [nc.dram_tensor(f"xs{i}", [P, XSLAB * P], bf16, kind="ExternalInput")
            for i in range(NXS)]
    W_d = nc.dram_tensor("W", [P, 136], bf16, kind="ExternalInput")
    ident_d = nc.dram_tensor("ident", [144, 32], i64, kind="ExternalInput")
    pidx_d = nc.dram_tensor("pidx", [P, IWPAD], i16, kind="ExternalInput")
    tidx_d = nc.dram_tensor("tidx", [P, IWPAD], i16, kind="ExternalInput")
    sidx_d = nc.dram_tensor("sidx", [P, IWPAD], i16, kind="ExternalInput")
    seq_d = nc.dram_tensor("seq", [P, 16], i16, kind="ExternalInput")
    if with_bias:
        bias_d = nc.dram_tensor("bias", [1, D], f32, kind="ExternalInput")
    out_d = nc.dram_tensor("out", [NBLK * P, D], f32, kind="ExternalOutput")

    # ---- tables (device-built) ----
    pt_lo = nc.dram_tensor("pt_lo", [ROWS_LO, 32], i64)
    pt_hi = nc.dram_tensor("pt_hi", [ROWS_HI, 32], i64)
    st_lo = nc.dram_tensor("st_lo", [ROWS_LO, 32], i64)
    st_hi = nc.dram_tensor("st_hi", [ROWS_HI, 32], i64)

    with tile.TileContext(nc) as tc:
        with (
            tc.tile_pool(name="const", bufs=1) as cp,
            tc.tile_pool(name="p1x", bufs=3) as p1x,
            tc.tile_pool(name="p1w", bufs=2) as p1w,
            tc.tile_pool(name="p1ps", bufs=2, space="PSUM") as p1ps,
            tc.tile_pool(name="g", bufs=CFG["g_bufs"]) as g,
            tc.tile_pool(name="wk", bufs=CFG["wk_bufs"]) as wk,
            tc.tile_pool(name="acc", bufs=CFG["acc_bufs"], space="PSUM") as accp,
            tc.tile_pool(name="ep", bufs=3) as ep,
        ):
            from concourse.library_config import mlp
            nc.gpsimd.load_library(mlp)

            seq = cp.tile([P, 16], i16)
            nc.sync.dma_start(seq[:], seq_d[:])
            W_sb = cp.tile([P, 136], bf16)
            nc.sync.dma_start(W_sb[:], W_d[:])
            nshift = cp.tile([P, 1], f32)
            nc.gpsimd.memset(nshift[:], -SHIFT)
            sc02 = cp.tile([P, 1], f32)
            nc.gpsimd.memset(sc02[:], 0.2)

            def bulk_load(dst_ap, src_t, n_i64):
                # dst[p, :] = src_t[p, :] via 128-row sequential gather
                nc.gpsimd.dma_gather(
                    dst_ap.rearrange("p (k c) -> p k c", k=1),
                    src_t, seq[:, :8], P, P, n_i64, single_packet=False)

            pidx = cp.tile([P, IWPAD], i16)
            tidx = cp.tile([P, IWPAD], i16)
            sidx = cp.tile([P, IWPAD], i16)
            bulk_load(pidx[:].bitcast(i64), pidx_d[:].bitcast(i64), IWPAD // 4)
            bulk_load(tidx[:].bitcast(i64), tidx_d[:].bitcast(i64), IWPAD // 4)
            bulk_load(sidx[:].bitcast(i64), sidx_d[:].bitcast(i64), IWPAD // 4)
            if with_bias:
                ones_row = cp.tile([1, P], f32)
                nc.gpsimd.memset(ones_row[:], 1.0)
                bias_row = cp.tile([1, D], f32)
                nc.sync.dma_start(bias_row[:], bias_d[:])
                bias_ps = accp.tile([P, D], f32, tag="init")
                nc.tensor.matmul(out=bias_ps[:], lhsT=ones_row[:], rhs=bias_row[:],
                                 start=True, stop=True)
                bias_mat = cp.tile([P, D], f32)
                nc.vector.tensor_copy(out=bias_mat[:], in_=bias_ps[:])

            # ---- phase 1: projection + scores -> tables ----
            xs = []
            for i in range(NXS):
                xt = p1x.tile([P, XSLAB * P], bf16, tag="xs")
                bulk_load(xt[:].bitcast(i64), xs_d[i][:].bitcast(i64),
                          XSLAB * P // 4)
                xs.append(xt)

            n_wslab = cdiv(NT, WSLAB)
            gi = 0
            for ws in range(n_wslab):
                base = ws * WSLAB
                w = min(WSLAB, NT - base)
                prow = p1w.tile([P, WSLAB * D], bf16, tag="prow")
                srow = p1w.tile([P, WSLAB * 8], f32, tag="srow")
                prow_r = prow[:].rearrange("p (j c) -> p j c", j=WSLAB)
                srow_r = srow[:].rearrange("p (j c) -> p j c", j=WSLAB)
                for g0 in range(0, w, PK):
                    k = min(PK, w - g0)
                    ps = p1ps.tile([P, PK * 256], f32, tag="p1")
                    for j in range(k):
                        nt = base + g0 + j
                        xt = xs[nt // XSLAB]
                        o = (nt % XSLAB) * P
                        nc.tensor.matmul(out=ps[:, j * 256:j * 256 + 136],
                                         lhsT=xt[:, o:o + P],
                                         rhs=W_sb[:], start=True, stop=True)
                    ps_r = ps[:].rearrange("p (j c) -> p j c", j=PK)[:, :k, :]
                    ceng = nc.scalar if (gi % 3) < CFG["p1_copy_act"] else nc.vector
                    gi += 1
                    if ceng is nc.scalar:
                        nc.scalar.activation(
                            out=prow_r[:, g0:g0 + k, :], in_=ps_r[:, :, 0:D],
                            func=mybir.ActivationFunctionType.Copy)
                    else:
                        nc.vector.tensor_copy(
                            out=prow_r[:, g0:g0 + k, :], in_=ps_r[:, :, 0:D])
                    nc.vector.tensor_copy(
                        out=srow_r[:, g0:g0 + k, :], in_=ps_r[:, :, D:D + 8])
                pr = prow_r[:, :w, :]
                sr = srow_r[:, :w, :]
                nc.gpsimd.dma_start(
                    pt_lo[:].bitcast(bf16).rearrange(
                        "(p nt) c -> p nt c", p=PSPLIT)[:, base:base + w, :],
                    pr[0:PSPLIT])
                nc.gpsimd.dma_start(
                    pt_hi[:].bitcast(bf16).rearrange(
                        "(p nt) c -> p nt c", p=P - PSPLIT)[:, base:base + w, :],
                    pr[PSPLIT:P])
                nc.sync.dma_start(
                    st_lo[:].bitcast(f32).rearrange(
                        "(p nt) c -> p nt c", p=PSPLIT)[:, base:base + w, 0:8],
                    sr[0:PSPLIT])
                nc.sync.dma_start(
                    st_hi[:].bitcast(f32).rearrange(
                        "(p nt) c -> p nt c", p=P - PSPLIT)[:, base:base + w, 0:8],
                    sr[PSPLIT:P])

            # ---- phase 2: per 128-target block ----
            for b in range(NBLK):
                st_t = st_lo if b < nb_lo else st_hi
                gi0 = b * IW
                rows = g.tile([P, T_B * 32], i64, tag="grow")
                srcs = g.tile([P, T_B * 32], i64, tag="gsrc")
                taut = g.tile([P, T_B * 32], i64, tag="gtau")
                Sg = g.tile([P, T_B * 32], i64, tag="gS")
                if k_plo:
                    nc.gpsimd.dma_gather(
                        rows[:, :k_plo * 32].rearrange("p (k c) -> p k c", k=k_plo),
                        pt_lo[:], pidx[:, gi0:gi0 + k_plo * 8],
                        k_plo * P, k_plo * P, 32, single_packet=False)
                    nc.gpsimd.dma_gather(
                        srcs[:, :k_plo * 32].rearrange("p (k c) -> p k c", k=k_plo),
                        st_lo[:], pidx[:, gi0:gi0 + k_plo * 8],
                        k_plo * P, k_plo * P, 32, single_packet=False)
                if k_phi:
                    nc.gpsimd.dma_gather(
                        rows[:, k_plo * 32:].rearrange("p (k c) -> p k c", k=k_phi),
                        pt_hi[:], pidx[:, gi0 + k_plo * 8:gi0 + IW],
                        k_phi * P, k_phi * P, 32, single_packet=False)
                    nc.gpsimd.dma_gather(
                        srcs[:, k_plo * 32:].rearrange("p (k c) -> p k c", k=k_phi),
                        st_hi[:], pidx[:, gi0 + k_plo * 8:gi0 + IW],
                        k_phi * P, k_phi * P, 32, single_packet=False)
                nc.gpsimd.dma_gather(
                    taut[:].rearrange("p (k c) -> p k c", k=T_B),
                    st_t[:], tidx[:, gi0:gi0 + IW],
                    NIDX, NIDX, 32, single_packet=False)
                nc.gpsimd.dma_gather(
                    Sg[:].rearrange("p (k c) -> p k c", k=T_B),
                    ident_d[:], sidx[:, gi0:gi0 + IW],
                    NIDX, NIDX, 32, single_packet=False)

                srcs_f = srcs[:].bitcast(f32).rearrange("p (j c) -> p j c", j=T_B)
                taut_f = taut[:].bitcast(f32).rearrange("p (j c) -> p j c", j=T_B)
                xb = wk.tile([P, T_B * H], f32, tag="xb")
                xb_r = xb[:].rearrange("p (j h) -> p j h", j=T_B)
                nc.vector.tensor_tensor(
                    out=xb_r, in0=srcs_f[:, :, 0:H], in1=taut_f[:, :, H:2 * H],
                    op=mybir.AluOpType.add)
                wide = wk.tile([P, T_B * (D + H)], bf16, tag="wide")
                wide_r = wide[:].rearrange("p (j c) -> p j c", j=T_B)
                e2 = wk.tile([P, T_B * H], bf16, tag="e2")
                e2_r = e2[:].rearrange("p (j h) -> p j h", j=T_B)
                nc.scalar.activation(
                    out=wide_r[:, :, D:], in_=xb_r,
                    func=mybir.ActivationFunctionType.Exp, bias=nshift[:])
                nc.scalar.activation(
                    out=e2_r, in_=xb_r,
                    func=mybir.ActivationFunctionType.Exp, bias=nshift[:],
                    scale=sc02[:])
                nc.vector.tensor_tensor(
                    out=wide_r[:, :, D:], in0=wide_r[:, :, D:], in1=e2_r,
                    op=mybir.AluOpType.max)
                nc.vector.tensor_tensor(
                    out=wide_r[:, :, :D].rearrange("p j (r h) -> p j r h", h=H),
                    in0=rows[:].bitcast(bf16).rearrange("p (j c) -> p j c", j=T_B)
                        .rearrange("p j (r h) -> p j r h", h=H),
                    in1=wide_r[:, :, D:].unsqueeze(2).to_broadcast([P, T_B, 32, H]),
                    op=mybir.AluOpType.mult)
                acc = accp.tile([P, D + H], f32, tag="acc")
                Sg_b = Sg[:].bitcast(bf16).rearrange("p (j c) -> p j c", j=T_B)
                for j in range(T_B):
                    nc.tensor.matmul(
                        out=acc[:], lhsT=Sg_b[:, j, :],
                        rhs=wide[:, j * (D + H):(j + 1) * (D + H)],
                        start=(j == 0), stop=(j == T_B - 1))
                den = ep.tile([P, H], f32, tag="den")
                nc.scalar.activation(out=den[:], in_=acc[:, D:],
                                     func=mybir.ActivationFunctionType.Copy,
                                     bias=float(EPS))
                recip = ep.tile([P, H], f32, tag="recip")
                nc.vector.reciprocal(recip[:], den[:])
                out_sb = ep.tile([P, D], f32, tag="outsb")
                nc.vector.tensor_tensor(
                    out=out_sb[:].rearrange("p (h r) -> p r h", h=H),
                    in0=acc[:, :D].rearrange("p (r h) -> p r h", h=H),
                    in1=recip[:].unsqueeze(1).to_broadcast([P, 32, H]),
                    op=mybir.AluOpType.mult)
                if with_bias:
                    nc.vector.tensor_tensor(
                        out=out_sb[:], in0=out_sb[:], in1=bias_mat[:],
                        op=mybir.AluOpType.add)
                nc.sync.dma_start(out_d[b * P:(b + 1) * P, :], out_sb[:])

    nc.compile()
    return nc


def _wrap16(seg):
    """dma_gather idx layout: entry i at [i%16, i//16], replicated x8."""
    n = len(seg)
    w = seg.reshape(n // 16, 16).T
    return np.tile(w, (8, 1))


def _prep_host(in_feat, edge_ind, W_proj, a_src, a_tgt, bias):
    import ml_dtypes
    bfd = ml_dtypes.bfloat16
    src = np.asarray(edge_ind[0]).astype(np.int64)
    tgt = np.asarray(edge_ind[1]).astype(np.int64)
    x = np.asarray(in_feat, np.float32)
    W = np.asarray(W_proj, np.float32)
    a_src = np.asarray(a_src, np.float32).reshape(H, 32)
    a_tgt = np.asarray(a_tgt, np.float32).reshape(H, 32)
    bias = np.asarray(bias, np.float32).reshape(-1)

    # W_ext: [W head-interleaved (col r*4+h) | W@a_src_h | W@a_tgt_h], bf16
    Wb = W.astype(bfd).astype(np.float32)
    perm = np.arange(D).reshape(H, 32).T.reshape(-1)   # new col r*4+h = old h*32+r
    W_ext = np.zeros((P, 136), np.float32)
    W_ext[:, :D] = Wb[:, perm]
    for h in range(H):
        sel = np.zeros((D,), np.float32)
        sel[h * 32:(h + 1) * 32] = a_src[h]
        W_ext[:, D + h] = Wb @ sel
        sel = np.zeros((D,), np.float32)
        sel[h * 32:(h + 1) * 32] = a_tgt[h]
        W_ext[:, D + H + h] = Wb @ sel

    xT = np.zeros((P, NPAD), np.float32)
    xT[:, :N_NODES] = x.T
    xs_in = {}
    for i in range(NXS):
        sl = np.zeros((P, XSLAB * P), bfd)
        w = min(XSLAB * P, NPAD - i * XSLAB * P)
        sl[:, :w] = xT[:, i * XSLAB * P:i * XSLAB * P + w].astype(bfd)
        xs_in[f"xs{i}"] = sl

    ident = np.zeros((144, P), bfd)
    for q in range(P):
        ident[q, q] = 1.0

    # ---- edge partitioning ----
    core = tgt // TPC
    p_of_t = tgt % P
    t_is_lo = p_of_t < PSPLIT
    src_is_lo = (src % P) < PSPLIT

    deg_lo = np.bincount(tgt[src_is_lo], minlength=N_NODES)
    deg_hi = np.bincount(tgt[~src_is_lo], minlength=N_NODES)
    blk_of = np.full(N_NODES, -1, np.int32)
    tin_of = np.zeros(N_NODES, np.int32)
    nb_lo = nb_hi = 0
    for c in range(N_CORES):
        ids_all = np.arange(c * TPC, (c + 1) * TPC)
        nb_lo = max(nb_lo, cdiv(int(((ids_all % P) < PSPLIT).sum()), P))
        nb_hi = max(nb_hi, cdiv(int(((ids_all % P) >= PSPLIT).sum()), P))
    for c in range(N_CORES):
        ids_all = np.arange(c * TPC, (c + 1) * TPC)
        for half, nb, b0 in ((0, nb_lo, 0), (1, nb_hi, nb_lo)):
            sel = (ids_all % P) < PSPLIT if half == 0 else (ids_all % P) >= PSPLIT
            ids = ids_all[sel]
            order = np.argsort(-(deg_lo[ids] + deg_hi[ids]), kind="stable")
            loads_l = np.zeros(nb, np.int64)
            loads_h = np.zeros(nb, np.int64)
            fill = np.zeros(nb, np.int32)
            for t in ids[order]:
                cand = np.nonzero(fill < P)[0]
                j = cand[np.argmin(np.maximum(loads_l[cand] + deg_lo[t],
                                              loads_h[cand] + deg_hi[t])
                                   + 0.001 * fill[cand])]
                blk_of[t] = b0 + j
                tin_of[t] = fill[j]
                fill[j] += 1
                loads_l[j] += deg_lo[t]
                loads_h[j] += deg_hi[t]
    NBLK = nb_lo + nb_hi
    blk = blk_of[tgt]
    tin = tin_of[tgt]

    key = core * NBLK + blk
    n_lo_e = np.bincount(key[src_is_lo], minlength=N_CORES * NBLK)
    n_hi_e = np.bincount(key[~src_is_lo], minlength=N_CORES * NBLK)
    k_plo = max(1, cdiv(int(n_lo_e.max()), P))
    k_phi = max(1, cdiv(int(n_hi_e.max()), P))
    T_B = k_plo + k_phi
    IW = T_B * 8
    IWPAD = cdiv(NBLK * IW * 2, 256) * 128

    prow_id = (src % P - np.where(src_is_lo, 0, PSPLIT)) * NT + src // P
    trow_id = (tgt % P - np.where(t_is_lo, 0, PSPLIT)) * NT + tgt // P

    seq = _wrap16(np.concatenate([np.arange(P, dtype=np.int16),
                                  np.zeros(P, np.int16)]))[:, :16]
    with_bias = bool(np.any(bias != 0.0))
    shared = {**xs_in, "W": W_ext.astype(bfd), "ident": ident.view(np.int64),
              "seq": seq}
    if with_bias:
        shared["bias"] = bias.reshape(1, D)

    core_inputs = []
    out_perm = np.full((N_CORES, NBLK * P), -1, np.int64)
    for c in range(N_CORES):
        ids_all = np.arange(c * TPC, (c + 1) * TPC)
        for t in ids_all:
            out_perm[c, blk_of[t] * P + tin_of[t]] = t
        m = core == c
        cs_p, cb, ct = prow_id[m], blk[m], tin[m]
        ct_row = trow_id[m]
        clo = src_is_lo[m]
        pidx = np.zeros((NBLK, T_B * P), np.int16)
        t16 = np.zeros((NBLK, T_B * P), np.int16)
        s16 = np.full((NBLK, T_B * P), 128, np.int16)   # pad -> zero one-hot row
        for b in range(NBLK):
            mb = cb == b
            lo_sel = mb & clo
            hi_sel = mb & ~clo
            nl, nh = int(lo_sel.sum()), int(hi_sel.sum())
            pidx[b, :nl] = cs_p[lo_sel].astype(np.int16)
            pidx[b, k_plo * P:k_plo * P + nh] = cs_p[hi_sel].astype(np.int16)
            t16[b, :nl] = ct_row[lo_sel].astype(np.int16)
            t16[b, k_plo * P:k_plo * P + nh] = ct_row[hi_sel].astype(np.int16)
            s16[b, :nl] = ct[lo_sel].astype(np.int16)
            s16[b, k_plo * P:k_plo * P + nh] = ct[hi_sel].astype(np.int16)
        pw = np.zeros((P, IWPAD), np.int16)
        tw = np.zeros((P, IWPAD), np.int16)
        sw = np.zeros((P, IWPAD), np.int16)
        for b in range(NBLK):
            pw[:, b * IW:b * IW + k_plo * 8] = _wrap16(pidx[b, :k_plo * P])
            pw[:, b * IW + k_plo * 8:(b + 1) * IW] = _wrap16(pidx[b, k_plo * P:])
            tw[:, b * IW:(b + 1) * IW] = _wrap16(t16[b])
            sw[:, b * IW:(b + 1) * IW] = _wrap16(s16[b])
        core_inputs.append({**shared, "pidx": pw, "tidx": tw, "sidx": sw})
    return (nb_lo, nb_hi, k_plo, k_phi, with_bias), core_inputs, out_perm


def kernel(in_feat, edge_ind, edge_len, W_proj, a_src, a_tgt, bias):
    kkey, core_inputs, out_perm = _prep_host(in_feat, edge_ind, W_proj,
                                             a_src, a_tgt, bias)
    if kkey not in _cache:
        _cache[kkey] = _build(*kkey)
    nc = _cache[kkey]

    from concourse.bass_utils import run_bass_kernel_spmd
    res = run_bass_kernel_spmd(nc, core_inputs, list(range(N_CORES)))

    out = np.zeros((N_NODES, D), np.float32)
    for c in range(N_CORES):
        o = res.results[c]["out"]
        valid = out_perm[c] >= 0
        out[out_perm[c][valid]] = o[valid]
    return out
